# revision 2
# baseline (speedup 1.0000x reference)
"""Trainium2 Bass kernel for nn_CenterLossNet (center-loss softmax over classes).

Math (reference):
    f = l2_normalize(features); c = l2_normalize(centers)
    dis[n,k]  = -5 * (|f_n|^2 + |c_k|^2 - 2 f_n.c_k)        # [N, C]
    pos[n]    = dis[n, labels[n]] + bias[labels[n]]
    den[n]    = sum_k exp(dis[n,k]) - exp(dis[n,l_n]) + exp(pos[n])
    loss      = mean(log(den) - pos) + var(pos, ddof=1);  returns (loss, var)

Device does the heavy part: S = f_hat @ c_hat.T (8192x10000x512 matmul) in
fp8e4m3 DoubleRow perf mode, fused with exp + row-sum of exp(10*S + bias_n).
The PE fills 2048-wide PSUM megatiles; drains alternate between two engines
so neither ever paces the PE:
  - ACT tiles: scalar-engine EXP with accum_out (exp + row-sum in one
    ACTIVATE, ~2us per megatile).
  - SCH tiles: Schraudolph bit-trick exp on the vector engine --
    code = int32(psum*A + B) is exp(dis) in float bits; one tensor_scalar
    (mult+add, f32->int32) plus one tensor_reduce over the bitcast-f32 tile.
    ~3% deterministic ripple, mean bias corrected on host per column.
Everything O(N) or O(C) runs on host in fp64, so pos/variance are exact.

Sharding: data-parallel over batch N across 8 cores; centers replicated.
Per-class |c_k|^2 is folded as exactly 1.0 into the row bias; the host
applies the mean residual correction (exp(-5*(c2-1)) averaged over C).
"""

import numpy as np
import ml_dtypes

import concourse.bacc as bacc
import concourse.mybir as mybir
import concourse.tile as tile
from concourse.bass_utils import run_bass_kernel_spmd

N, C, D = 8192, 10000, 512
N_CORES = 8
NS = N // N_CORES       # 1024 rows per core
P = 128                 # partitions
M_TILES = NS // P       # 8 row tiles per core
K2 = D // (2 * P)       # 2 DoubleRow contraction tiles (256 rows each)
CW = 512                # matmul free-dim tile (one PSUM bank of fp32)
GW = 2048               # PSUM megatile width: 4 banks
G_TILES = (C + GW - 1) // GW  # 5 (4 x 2048 + 1808)
N_TILES = G_TILES * M_TILES   # 40 megatiles per core
SCALE = 5.0
EPS = 1e-12
FP8_SCALE = 512.0       # 2^9: keeps |values| <= ~120 within e4m3 normal range
FP8 = ml_dtypes.float8_e4m3

# Schraudolph exp: exp(x) ~= bitcast_f32(int32(x*SCH_A + SCH_B))
SCH_A = 8388608.0 / np.log(2.0)          # 2^23 / ln2
SCH_C = 366393.0                          # minimizes max rel err (~3%)
SCH_B = 127.0 * 8388608.0 - SCH_C

# Megatile drain assignment: tile 0 ACT (primes the exp table early, split in
# halves), odd tiles >=3 ACT (so the last tile 39 finishes on a fused
# ACT accum, no trailing vector work), the rest Schraudolph on DVE.
ACT_TILES = [0] + [t for t in range(3, N_TILES, 2)]
SCH_TILES = [t for t in range(N_TILES) if t not in ACT_TILES]
ACT_POS = {t: i for i, t in enumerate(ACT_TILES)}
SCH_POS = {t: i for i, t in enumerate(SCH_TILES)}
N_ACT_COLS = len(ACT_TILES) + 1          # tile 0 uses two accum slots
N_SCH_COLS = len(SCH_TILES)

_compiled = None
LAST_RESULTS = None


def _build():
    nc = bacc.Bacc(
        "TRN2",
        target_bir_lowering=False,
        debug=False,
        enable_asserts=False,
        num_devices=N_CORES,
    )
    # strip-major: per strip each partition's row is 8 KB contiguous in DRAM;
    # strip 0 is stored as two contiguous half-strips for a fast first fill
    ct0_d = nc.dram_tensor(
        "ct0", [2, P, K2, 2, GW // 2], mybir.dt.float8e4, kind="ExternalInput"
    ).ap()
    ct_d = nc.dram_tensor(
        "ct", [G_TILES - 1, P, K2, 2, GW], mybir.dt.float8e4, kind="ExternalInput"
    ).ap()
    ft_d = nc.dram_tensor(
        "ft", [P, K2, 2, NS], mybir.dt.float8e4, kind="ExternalInput"
    ).ap()
    ab_d = nc.dram_tensor("ab", [P, M_TILES], mybir.dt.float32, kind="ExternalInput").ap()
    # Schraudolph-transformed bias: ab*SCH_A + SCH_B, per row tile
    ab2_d = nc.dram_tensor(
        "ab2", [P, M_TILES], mybir.dt.float32, kind="ExternalInput"
    ).ap()
    # per-(g,m) partial row-sums; first N_ACT_COLS are ACT partials, then SCH
    rs_d = nc.dram_tensor(
        "rs", [P, N_ACT_COLS + N_SCH_COLS], mybir.dt.float32, kind="ExternalOutput"
    ).ap()

    with tile.TileContext(nc) as tc:
        with (
            tc.tile_pool(name="cpool", bufs=1) as cpool,
            tc.tile_pool(name="fpool", bufs=1) as fpool,
            tc.tile_pool(name="spool", bufs=1) as spool,
            tc.tile_pool(name="epool", bufs=3) as epool,
            tc.tile_pool(name="ipool", bufs=3) as ipool,
            tc.tile_pool(name="partpool", bufs=1) as partpool,
            tc.tile_pool(name="ppool", bufs=2, space="PSUM") as ppool,
        ):
            # warm the PE clock (HAM) with throwaway DoubleRow matmuls on a
            # zeroed tile while the first input DMAs are still in flight
            z8 = spool.tile([P, 2, CW], mybir.dt.float8e4, tag="z8")
            nc.gpsimd.memset(z8[:], 0.0)
            wps = ppool.tile([P, CW], mybir.dt.float32, tag="ps", name="wps")
            for _ in range(20):
                nc.tensor.matmul(
                    wps[:],
                    z8[:, :, 0:P],
                    z8[:],
                    start=True,
                    stop=True,
                    perf_mode=mybir.MatmulPerfMode.DoubleRow,
                    skip_group_check=True,
                )

            # critical prefix on the fast sync ring, in first-use order
            # (bias is tiny and gates every ACTIVATE - it goes first)
            bias_sb = spool.tile([P, M_TILES], mybir.dt.float32, tag="bias")
            nc.sync.dma_start(out=bias_sb[:], in_=ab_d)
            bias2_sb = spool.tile([P, M_TILES], mybir.dt.float32, tag="bias2")
            nc.sync.dma_start(out=bias2_sb[:], in_=ab2_d)

            ct0a = cpool.tile([P, K2, 2, GW // 2], mybir.dt.float8e4, tag="ct0a")
            nc.sync.dma_start(out=ct0a[:], in_=ct0_d[0])

            ft_sb = fpool.tile([P, K2, 2, NS], mybir.dt.float8e4, tag="ft")
            nc.sync.dma_start(out=ft_sb[:], in_=ft_d)

            ct0b = cpool.tile([P, K2, 2, GW // 2], mybir.dt.float8e4, tag="ct0b")
            nc.sync.dma_start(out=ct0b[:], in_=ct0_d[1])

            # remaining strips: one DMA per strip, all FIFO on the sync ring
            # so late strips never contend with the critical prefix
            ct_sb = [None]
            for g in range(1, G_TILES):
                gw = min(GW, C - g * GW)
                t = cpool.tile(
                    [P, K2, 2, GW], mybir.dt.float8e4, tag=f"ct{g}", name=f"ct{g}"
                )
                nc.sync.dma_start(out=t[:, :, :, :gw], in_=ct_d[g - 1][:, :, :, :gw])
                ct_sb.append(t)

            parts_act = partpool.tile([P, N_ACT_COLS], mybir.dt.float32, tag="pa")
            parts_sch = partpool.tile([P, N_SCH_COLS], mybir.dt.float32, tag="pd")

            # strip-outer / row-tile-inner: PE is dense as soon as strip 0 lands
            for g in range(G_TILES):
                gw = min(GW, C - g * GW)
                n_sl = (gw + CW - 1) // CW
                for m in range(M_TILES):
                    tile_idx = g * M_TILES + m
                    on_act = tile_idx in ACT_POS
                    ps = ppool.tile([P, GW], mybir.dt.float32, tag="ps")
                    # first megatile: finish columns in halves (j-outer) and
                    # exp each half as it lands, so ACT primes ~3us earlier
                    split_act = tile_idx == 0
                    kj = (
                        [(k, j) for j in range(n_sl) for k in range(K2)]
                        if split_act
                        else [(k, j) for k in range(K2) for j in range(n_sl)]
                    )
                    for k, j in kj:
                        w = min(CW, gw - j * CW)
                        if g == 0:
                            half = (j * CW) // (GW // 2)
                            off = (j * CW) % (GW // 2)
                            rhs = (ct0a if half == 0 else ct0b)[
                                :, k, :, off : off + w
                            ]
                        else:
                            rhs = ct_sb[g][:, k, :, j * CW : j * CW + w]
                        nc.tensor.matmul(
                            ps[:, j * CW : j * CW + w],
                            ft_sb[:, k, :, m * P : (m + 1) * P],
                            rhs,
                            start=(k == 0),
                            stop=(k == K2 - 1),
                            perf_mode=mybir.MatmulPerfMode.DoubleRow,
                            skip_group_check=True,
                        )
                    if split_act:
                        # ACT path, two halves, each with its own accum slot
                        et = epool.tile([P, GW], mybir.dt.bfloat16, tag="exp")
                        h = GW // 2
                        for hi in range(2):
                            nc.scalar.activation(
                                et[:, hi * h : (hi + 1) * h],
                                ps[:, hi * h : (hi + 1) * h],
                                mybir.ActivationFunctionType.Exp,
                                bias=bias_sb[:, m : m + 1],
                                scale=2.0 * SCALE / (FP8_SCALE * FP8_SCALE),
                                accum_out=parts_act[:, hi : hi + 1],
                            )
                    elif on_act:
                        ci = ACT_POS[tile_idx] + 1
                        et = epool.tile([P, GW], mybir.dt.bfloat16, tag="exp")
                        nc.scalar.activation(
                            et[:, :gw],
                            ps[:, :gw],
                            mybir.ActivationFunctionType.Exp,
                            bias=bias_sb[:, m : m + 1],
                            scale=2.0 * SCALE / (FP8_SCALE * FP8_SCALE),
                            accum_out=parts_act[:, ci : ci + 1],
                        )
                    else:
                        # Schraudolph on DVE: int32(psum*A' + B_m), then
                        # row-sum of the bitcast-f32 codes
                        ci = SCH_POS[tile_idx]
                        it = ipool.tile([P, GW], mybir.dt.int32, tag="icode")
                        nc.vector.tensor_scalar(
                            it[:, :gw],
                            ps[:, :gw],
                            float(SCH_A * 2.0 * SCALE / (FP8_SCALE * FP8_SCALE)),
                            bias2_sb[:, m : m + 1],
                            op0=mybir.AluOpType.mult,
                            op1=mybir.AluOpType.add,
                        )
                        nc.vector.tensor_reduce(
                            parts_sch[:, ci : ci + 1],
                            it[:, :gw].bitcast(mybir.dt.float32),
                            axis=mybir.AxisListType.X,
                            op=mybir.AluOpType.add,
                        )
            nc.sync.dma_start(out=rs_d[:, 0:N_ACT_COLS], in_=parts_act[:])
            nc.sync.dma_start(out=rs_d[:, N_ACT_COLS:], in_=parts_sch[:])

    nc.compile()
    return nc


def _get_compiled():
    global _compiled
    if _compiled is None:
        _compiled = _build()
    return _compiled


def _l2n(x):
    n = np.sqrt(np.einsum("nd,nd->n", x, x, dtype=np.float32), dtype=np.float32)
    xh = x / np.maximum(n, np.float32(EPS))[:, None]
    sq = np.einsum("nd,nd->n", xh, xh, dtype=np.float32)
    return xh.astype(np.float32), sq.astype(np.float32)


def _pack_dr(xt):
    """[D, W] fp32 (pre-scaled) -> DoubleRow fp8 [P, K2, 2, W]:
    row d = k*256 + i*128 + p  ->  out[p, k, i]."""
    d, w = xt.shape
    return np.ascontiguousarray(
        xt.reshape(K2, 2, P, w).transpose(2, 0, 1, 3)
    ).astype(FP8)


def _pack_ct(xt):
    """[D, C] fp32 (pre-scaled) -> (strip-0 halves [2, P, K2, 2, GW/2],
    strips 1.. [G-1, P, K2, 2, GW], last zero-padded)."""
    ct0 = np.stack(
        [_pack_dr(xt[:, 0 : GW // 2]), _pack_dr(xt[:, GW // 2 : GW])]
    )
    ctr = np.zeros((G_TILES - 1, P, K2, 2, GW), dtype=FP8)
    for g in range(1, G_TILES):
        gw = min(GW, C - g * GW)
        ctr[g - 1, :, :, :, :gw] = _pack_dr(xt[:, g * GW : g * GW + gw])
    return ct0, ctr


def _sch_emulate(x):
    """Numpy emulation of the device Schraudolph path for fp32 input x."""
    code = (np.float32(x) * np.float32(SCH_A) + np.float32(SCH_B)).astype(np.int32)
    return code.view(np.float32)


def _sch_mean_corr():
    """Mean multiplicative bias of the Schraudolph approx over a uniform
    phase (inputs spread over many ln2 periods), to divide out on host."""
    x = np.linspace(-12.0, -12.0 + np.log(2.0), 65537, dtype=np.float64)[:-1]
    ratio = _sch_emulate(x.astype(np.float32)).astype(np.float64) / np.exp(x)
    return ratio.mean()


SCH_CORR = 1.0 / _sch_mean_corr()


def _combine_rs(rs):
    """[P, N_ACT_COLS+N_SCH_COLS] per-core output -> per-row sums [NS]
    (n = m*128 + p). Applies the Schraudolph mean-bias correction to SCH
    columns and re-interleaves storage order back to tile order."""
    rs = rs.astype(np.float64)
    vals = np.empty((P, N_TILES), dtype=np.float64)
    # tile 0 = sum of the two half-accums
    vals[:, 0] = rs[:, 0] + rs[:, 1]
    for t in ACT_TILES[1:]:
        vals[:, t] = rs[:, ACT_POS[t] + 1]
    for t in SCH_TILES:
        vals[:, t] = rs[:, N_ACT_COLS + SCH_POS[t]] * SCH_CORR
    out = vals.reshape(P, G_TILES, M_TILES).sum(axis=1)
    return out.T.reshape(NS)


def kernel(features, labels, centers, bias):
    features = np.asarray(features, dtype=np.float32)
    centers = np.asarray(centers, dtype=np.float32)
    bias = np.asarray(bias, dtype=np.float32)
    labels_i = np.asarray(labels).astype(np.int64)

    fh, f2 = _l2n(features)          # [N, D], [N]
    ch, c2 = _l2n(centers)           # [C, D], [C]

    ct0_8, ct8 = _pack_ct(ch.T * np.float32(FP8_SCALE))
    abias_full = (-SCALE * (f2 + np.float32(1.0))).astype(np.float32)
    ab2_full = (
        abias_full.astype(np.float64) * SCH_A + SCH_B
    ).astype(np.float32)

    in_maps = []
    for i in range(N_CORES):
        sl = slice(i * NS, (i + 1) * NS)
        ft8 = _pack_dr(fh[sl].T * np.float32(FP8_SCALE))    # [P, K2, 2, NS]
        ab = np.ascontiguousarray(
            abias_full[sl].reshape(M_TILES, P).T
        )  # [P, M_TILES], n = m*128 + p
        ab2 = np.ascontiguousarray(ab2_full[sl].reshape(M_TILES, P).T)
        in_maps.append({"ct0": ct0_8, "ct": ct8, "ft": ft8, "ab": ab, "ab2": ab2})

    nc = _get_compiled()
    global LAST_RESULTS
    LAST_RESULTS = run_bass_kernel_spmd(nc, in_maps, core_ids=list(range(N_CORES)))

    rowsum = np.concatenate(
        [_combine_rs(LAST_RESULTS.results[i]["rs"]) for i in range(N_CORES)]
    ).astype(np.float64)

    # residual correction for the |c_k|^2 ~= 1 fold (mean of exp(-5*(c2-1)))
    wmean = np.exp(-SCALE * (c2.astype(np.float64) - 1.0)).mean()
    rowsum *= wmean

    # exact per-row label terms (fp32 inputs, fp64 math)
    cl = ch[labels_i]                                        # [N, D]
    dot = np.einsum("nd,nd->n", fh.astype(np.float64), cl.astype(np.float64))
    dis_l = -SCALE * (f2.astype(np.float64) + c2[labels_i].astype(np.float64) - 2.0 * dot)
    pos = dis_l + bias[labels_i, 0].astype(np.float64)

    num = np.exp(pos)
    den = rowsum - np.exp(dis_l) + num
    logits = np.log(den) - pos
    variance = np.var(pos, ddof=1)
    loss = logits.mean() + variance
    return (np.float32(loss), np.float32(variance))


# revision 13
# speedup vs baseline: 1.0247x; 1.0247x over previous
"""Trainium2 Bass kernel for nn_CenterLossNet (center-loss softmax over classes).

Math (reference):
    f = l2_normalize(features); c = l2_normalize(centers)
    dis[n,k]  = -5 * (|f_n|^2 + |c_k|^2 - 2 f_n.c_k)        # [N, C]
    pos[n]    = dis[n, labels[n]] + bias[labels[n]]
    den[n]    = sum_k exp(dis[n,k]) - exp(dis[n,l_n]) + exp(pos[n])
    loss      = mean(log(den) - pos) + var(pos, ddof=1);  returns (loss, var)

Device does the heavy part: S = f_hat @ c_hat.T (8192x10000x512 matmul) in
fp8e4m3 DoubleRow perf mode, fused with exp + row-sum of exp(10*S + bias_n).
The PE fills 2048-wide PSUM megatiles; drains alternate between two engines
so neither ever paces the PE:
  - ACT tiles: scalar-engine EXP with accum_out (exp + row-sum in one
    ACTIVATE, ~2us per megatile).
  - SCH tiles: Schraudolph bit-trick exp on the vector engine --
    code = int32(psum*A + B) is exp(dis) in float bits; one tensor_scalar
    (mult+add, f32->int32) plus one tensor_reduce over the bitcast-f32 tile.
    ~3% deterministic ripple, mean bias corrected on host per column.
Everything O(N) or O(C) runs on host in fp64, so pos/variance are exact.

Sharding: data-parallel over batch N across 8 cores; centers replicated.
Per-class |c_k|^2 is folded as exactly 1.0 into the row bias; the host
applies the mean residual correction (exp(-5*(c2-1)) averaged over C).
"""

import numpy as np
import ml_dtypes

import concourse.bacc as bacc
import concourse.mybir as mybir
import concourse.tile as tile
from concourse.bass_utils import run_bass_kernel_spmd

N, C, D = 8192, 10000, 512
N_CORES = 8
NS = N // N_CORES       # 1024 rows per core
P = 128                 # partitions
M_TILES = NS // P       # 8 row tiles per core
K2 = D // (2 * P)       # 2 DoubleRow contraction tiles (256 rows each)
CW = 512                # matmul free-dim tile (one PSUM bank of fp32)
GW = 2048               # PSUM megatile width: 4 banks
G_TILES = (C + GW - 1) // GW  # 5 (4 x 2048 + 1808)
N_TILES = G_TILES * M_TILES   # 40 megatiles per core
SCALE = 5.0
EPS = 1e-12
FP8_SCALE = 512.0       # 2^9: keeps |values| <= ~120 within e4m3 normal range
FP8 = ml_dtypes.float8_e4m3

# Schraudolph exp in bf16: exp(x) ~= bitcast_bf16(int16(x*SCH_A + SCH_B)).
# int16 codes let the DVE row-sum run at 2x (16-bit) rate.
SCH_A = 128.0 / np.log(2.0)              # 2^7 / ln2
SCH_C = 366393.0 / 65536.0                # minimizes max rel err (~3.5%)
SCH_B = 127.0 * 128.0 - SCH_C

# Megatile drain assignment: tile 0 ACT (primes the exp table early, split in
# halves), odd tiles >=3 ACT (so the last tile 39 finishes on a fused
# ACT accum, no trailing vector work), the rest Schraudolph on the DVE:
# f32->int16 convert, then a 2x-rate tensor_reduce of the bitcast-bf16 codes.
# (The Pool engine's ISA has no tensor_scalar-with-accum, so GpSimd can't
# take reduces.)
ACT_TILES = [0] + [t for t in range(3, N_TILES, 2)]
SCH_TILES = [t for t in range(N_TILES) if t not in ACT_TILES]
SCH_DVE = {t: i for i, t in enumerate(SCH_TILES)}         # reduce on DVE
ACT_POS = {t: i for i, t in enumerate(ACT_TILES)}
N_ACT_COLS = len(ACT_TILES) + 1          # tile 0 uses two accum slots
N_DVE_COLS = len(SCH_DVE)

_compiled = None
LAST_RESULTS = None


def _build():
    nc = bacc.Bacc(
        "TRN2",
        target_bir_lowering=False,
        debug=False,
        enable_asserts=False,
        num_devices=N_CORES,
    )
    # strip-major: per strip each partition's row is 8 KB contiguous in DRAM;
    # strip 0 is stored as two contiguous half-strips for a fast first fill
    ct0_d = nc.dram_tensor(
        "ct0", [2, P, K2, 2, GW // 2], mybir.dt.float8e4, kind="ExternalInput"
    ).ap()
    ct_d = nc.dram_tensor(
        "ct", [G_TILES - 1, P, K2, 2, GW], mybir.dt.float8e4, kind="ExternalInput"
    ).ap()
    ft_d = nc.dram_tensor(
        "ft", [P, K2, 2, NS], mybir.dt.float8e4, kind="ExternalInput"
    ).ap()
    ab_d = nc.dram_tensor("ab", [P, M_TILES], mybir.dt.float32, kind="ExternalInput").ap()
    # Schraudolph-transformed bias: ab*SCH_A + SCH_B, per row tile
    ab2_d = nc.dram_tensor(
        "ab2", [P, M_TILES], mybir.dt.float32, kind="ExternalInput"
    ).ap()
    # per-(g,m) partial row-sums: ACT partials first, then the DVE ones
    rs_d = nc.dram_tensor(
        "rs",
        [P, N_ACT_COLS + N_DVE_COLS],
        mybir.dt.float32,
        kind="ExternalOutput",
    ).ap()

    with tile.TileContext(nc) as tc:
        with (
            tc.tile_pool(name="cpool", bufs=1) as cpool,
            tc.tile_pool(name="fpool", bufs=1) as fpool,
            tc.tile_pool(name="spool", bufs=1) as spool,
            tc.tile_pool(name="epool", bufs=3) as epool,
            tc.tile_pool(name="ipool", bufs=3) as ipool,
            tc.tile_pool(name="partpool", bufs=1) as partpool,
            tc.tile_pool(name="ppool", bufs=2, space="PSUM") as ppool,
        ):
            # warm the PE clock (HAM) with throwaway DoubleRow matmuls on a
            # zeroed tile while the first input DMAs are still in flight
            z8 = spool.tile([P, 2, CW], mybir.dt.float8e4, tag="z8")
            nc.gpsimd.memset(z8[:], 0.0)
            wps = ppool.tile([P, CW], mybir.dt.float32, tag="ps", name="wps")
            for _ in range(20):
                nc.tensor.matmul(
                    wps[:],
                    z8[:, :, 0:P],
                    z8[:],
                    start=True,
                    stop=True,
                    perf_mode=mybir.MatmulPerfMode.DoubleRow,
                    skip_group_check=True,
                )

            # critical prefix on the fast sync ring, in first-use order
            # (bias is tiny and gates every ACTIVATE - it goes first)
            bias_sb = spool.tile([P, M_TILES], mybir.dt.float32, tag="bias")
            nc.sync.dma_start(out=bias_sb[:], in_=ab_d)
            bias2_sb = spool.tile([P, M_TILES], mybir.dt.float32, tag="bias2")
            nc.sync.dma_start(out=bias2_sb[:], in_=ab2_d)

            ct0a = cpool.tile([P, K2, 2, GW // 2], mybir.dt.float8e4, tag="ct0a")
            nc.sync.dma_start(out=ct0a[:], in_=ct0_d[0])

            ft_sb = fpool.tile([P, K2, 2, NS], mybir.dt.float8e4, tag="ft")
            nc.sync.dma_start(out=ft_sb[:], in_=ft_d)

            ct0b = cpool.tile([P, K2, 2, GW // 2], mybir.dt.float8e4, tag="ct0b")
            nc.sync.dma_start(out=ct0b[:], in_=ct0_d[1])

            # remaining strips: one DMA per strip, all FIFO on the sync ring
            # so late strips never contend with the critical prefix
            ct_sb = [None]
            for g in range(1, G_TILES):
                gw = min(GW, C - g * GW)
                t = cpool.tile(
                    [P, K2, 2, GW], mybir.dt.float8e4, tag=f"ct{g}", name=f"ct{g}"
                )
                nc.sync.dma_start(out=t[:, :, :, :gw], in_=ct_d[g - 1][:, :, :, :gw])
                ct_sb.append(t)

            parts_act = partpool.tile([P, N_ACT_COLS], mybir.dt.float32, tag="pa")
            parts_dve = partpool.tile([P, N_DVE_COLS], mybir.dt.float32, tag="pd")

            # strip-outer / row-tile-inner: PE is dense as soon as strip 0 lands
            for g in range(G_TILES):
                gw = min(GW, C - g * GW)
                n_sl = (gw + CW - 1) // CW
                for m in range(M_TILES):
                    tile_idx = g * M_TILES + m
                    on_act = tile_idx in ACT_POS
                    ps = ppool.tile([P, GW], mybir.dt.float32, tag="ps")
                    # first megatile: finish columns in halves (j-outer) and
                    # exp each half as it lands, so ACT primes ~3us earlier
                    split_act = tile_idx == 0
                    kj = (
                        [(k, j) for j in range(n_sl) for k in range(K2)]
                        if split_act
                        else [(k, j) for k in range(K2) for j in range(n_sl)]
                    )
                    for k, j in kj:
                        w = min(CW, gw - j * CW)
                        if g == 0:
                            half = (j * CW) // (GW // 2)
                            off = (j * CW) % (GW // 2)
                            rhs = (ct0a if half == 0 else ct0b)[
                                :, k, :, off : off + w
                            ]
                        else:
                            rhs = ct_sb[g][:, k, :, j * CW : j * CW + w]
                        nc.tensor.matmul(
                            ps[:, j * CW : j * CW + w],
                            ft_sb[:, k, :, m * P : (m + 1) * P],
                            rhs,
                            start=(k == 0),
                            stop=(k == K2 - 1),
                            perf_mode=mybir.MatmulPerfMode.DoubleRow,
                            skip_group_check=True,
                        )
                    if split_act:
                        # ACT path, two halves, each with its own accum slot
                        et = epool.tile([P, GW], mybir.dt.bfloat16, tag="exp")
                        h = GW // 2
                        for hi in range(2):
                            nc.scalar.activation(
                                et[:, hi * h : (hi + 1) * h],
                                ps[:, hi * h : (hi + 1) * h],
                                mybir.ActivationFunctionType.Exp,
                                bias=bias_sb[:, m : m + 1],
                                scale=2.0 * SCALE / (FP8_SCALE * FP8_SCALE),
                                accum_out=parts_act[:, hi : hi + 1],
                            )
                    elif on_act:
                        ci = ACT_POS[tile_idx] + 1
                        et = epool.tile([P, GW], mybir.dt.bfloat16, tag="exp")
                        nc.scalar.activation(
                            et[:, :gw],
                            ps[:, :gw],
                            mybir.ActivationFunctionType.Exp,
                            bias=bias_sb[:, m : m + 1],
                            scale=2.0 * SCALE / (FP8_SCALE * FP8_SCALE),
                            accum_out=parts_act[:, ci : ci + 1],
                        )
                    else:
                        # Schraudolph on DVE: int16(psum*A' + B_m) = exp in
                        # bf16 bits, then row-sum of the bitcast-bf16 codes
                        it = ipool.tile([P, GW], mybir.dt.int16, tag="icode")
                        nc.vector.tensor_scalar(
                            it[:, :gw],
                            ps[:, :gw],
                            float(SCH_A * 2.0 * SCALE / (FP8_SCALE * FP8_SCALE)),
                            bias2_sb[:, m : m + 1],
                            op0=mybir.AluOpType.mult,
                            op1=mybir.AluOpType.add,
                        )
                        ci = SCH_DVE[tile_idx]
                        nc.vector.tensor_reduce(
                            parts_dve[:, ci : ci + 1],
                            it[:, :gw].bitcast(mybir.dt.bfloat16),
                            axis=mybir.AxisListType.X,
                            op=mybir.AluOpType.add,
                        )
            nc.sync.dma_start(out=rs_d[:, 0:N_ACT_COLS], in_=parts_act[:])
            nc.sync.dma_start(out=rs_d[:, N_ACT_COLS:], in_=parts_dve[:])

    nc.compile()
    return nc


def _get_compiled():
    global _compiled
    if _compiled is None:
        _compiled = _build()
    return _compiled


def _l2n(x):
    n = np.sqrt(np.einsum("nd,nd->n", x, x, dtype=np.float32), dtype=np.float32)
    xh = x / np.maximum(n, np.float32(EPS))[:, None]
    sq = np.einsum("nd,nd->n", xh, xh, dtype=np.float32)
    return xh.astype(np.float32), sq.astype(np.float32)


def _pack_dr(xt):
    """[D, W] fp32 (pre-scaled) -> DoubleRow fp8 [P, K2, 2, W]:
    row d = k*256 + i*128 + p  ->  out[p, k, i]."""
    d, w = xt.shape
    return np.ascontiguousarray(
        xt.reshape(K2, 2, P, w).transpose(2, 0, 1, 3)
    ).astype(FP8)


def _pack_ct(xt):
    """[D, C] fp32 (pre-scaled) -> (strip-0 halves [2, P, K2, 2, GW/2],
    strips 1.. [G-1, P, K2, 2, GW], last zero-padded)."""
    ct0 = np.stack(
        [_pack_dr(xt[:, 0 : GW // 2]), _pack_dr(xt[:, GW // 2 : GW])]
    )
    ctr = np.zeros((G_TILES - 1, P, K2, 2, GW), dtype=FP8)
    for g in range(1, G_TILES):
        gw = min(GW, C - g * GW)
        ctr[g - 1, :, :, :, :gw] = _pack_dr(xt[:, g * GW : g * GW + gw])
    return ct0, ctr


def _sch_emulate(x):
    """Numpy emulation of the device Schraudolph path for fp32 input x."""
    code = (np.float32(x) * np.float32(SCH_A) + np.float32(SCH_B)).astype(np.int16)
    return code.view(ml_dtypes.bfloat16).astype(np.float32)


def _sch_mean_corr():
    """Mean multiplicative bias of the Schraudolph approx over a uniform
    phase (inputs spread over many ln2 periods), to divide out on host."""
    x = np.linspace(-12.0, -12.0 + np.log(2.0), 65537, dtype=np.float64)[:-1]
    ratio = _sch_emulate(x.astype(np.float32)).astype(np.float64) / np.exp(x)
    return ratio.mean()


SCH_CORR = 1.0 / _sch_mean_corr()


def _combine_rs(rs):
    """[P, N_ACT_COLS+N_GPS_COLS+N_DVE_COLS] per-core output -> per-row sums
    [NS] (n = m*128 + p). Applies the Schraudolph mean-bias correction to SCH
    columns and re-interleaves storage order back to tile order."""
    rs = rs.astype(np.float64)
    vals = np.empty((P, N_TILES), dtype=np.float64)
    # tile 0 = sum of the two half-accums
    vals[:, 0] = rs[:, 0] + rs[:, 1]
    for t in ACT_TILES[1:]:
        vals[:, t] = rs[:, ACT_POS[t] + 1]
    for t, i in SCH_DVE.items():
        vals[:, t] = rs[:, N_ACT_COLS + i] * SCH_CORR
    out = vals.reshape(P, G_TILES, M_TILES).sum(axis=1)
    return out.T.reshape(NS)


def kernel(features, labels, centers, bias):
    features = np.asarray(features, dtype=np.float32)
    centers = np.asarray(centers, dtype=np.float32)
    bias = np.asarray(bias, dtype=np.float32)
    labels_i = np.asarray(labels).astype(np.int64)

    fh, f2 = _l2n(features)          # [N, D], [N]
    ch, c2 = _l2n(centers)           # [C, D], [C]

    ct0_8, ct8 = _pack_ct(ch.T * np.float32(FP8_SCALE))
    abias_full = (-SCALE * (f2 + np.float32(1.0))).astype(np.float32)
    ab2_full = (
        abias_full.astype(np.float64) * SCH_A + SCH_B
    ).astype(np.float32)

    in_maps = []
    for i in range(N_CORES):
        sl = slice(i * NS, (i + 1) * NS)
        ft8 = _pack_dr(fh[sl].T * np.float32(FP8_SCALE))    # [P, K2, 2, NS]
        ab = np.ascontiguousarray(
            abias_full[sl].reshape(M_TILES, P).T
        )  # [P, M_TILES], n = m*128 + p
        ab2 = np.ascontiguousarray(ab2_full[sl].reshape(M_TILES, P).T)
        in_maps.append({"ct0": ct0_8, "ct": ct8, "ft": ft8, "ab": ab, "ab2": ab2})

    nc = _get_compiled()
    global LAST_RESULTS
    LAST_RESULTS = run_bass_kernel_spmd(nc, in_maps, core_ids=list(range(N_CORES)))

    rowsum = np.concatenate(
        [_combine_rs(LAST_RESULTS.results[i]["rs"]) for i in range(N_CORES)]
    ).astype(np.float64)

    # residual correction for the |c_k|^2 ~= 1 fold (mean of exp(-5*(c2-1)))
    wmean = np.exp(-SCALE * (c2.astype(np.float64) - 1.0)).mean()
    rowsum *= wmean

    # exact per-row label terms (fp32 inputs, fp64 math)
    cl = ch[labels_i]                                        # [N, D]
    dot = np.einsum("nd,nd->n", fh.astype(np.float64), cl.astype(np.float64))
    dis_l = -SCALE * (f2.astype(np.float64) + c2[labels_i].astype(np.float64) - 2.0 * dot)
    pos = dis_l + bias[labels_i, 0].astype(np.float64)

    num = np.exp(pos)
    den = rowsum - np.exp(dis_l) + num
    logits = np.log(den) - pos
    variance = np.var(pos, ddof=1)
    loss = logits.mean() + variance
    return (np.float32(loss), np.float32(variance))


# revision 16
# speedup vs baseline: 1.2199x; 1.1905x over previous
"""Trainium2 Bass kernel for nn_CenterLossNet (center-loss softmax over classes).

Math (reference):
    f = l2_normalize(features); c = l2_normalize(centers)
    dis[n,k]  = -5 * (|f_n|^2 + |c_k|^2 - 2 f_n.c_k)        # [N, C]
    pos[n]    = dis[n, labels[n]] + bias[labels[n]]
    den[n]    = sum_k exp(dis[n,k]) - exp(dis[n,l_n]) + exp(pos[n])
    loss      = mean(log(den) - pos) + var(pos, ddof=1);  returns (loss, var)

Device does the heavy part: S = f_hat @ c_hat.T (8192x10000x512 matmul) in
fp8e4m3 DoubleRow perf mode, fused with exp + row-sum of exp(10*S + bias_n).
The PE fills 2048-wide PSUM megatiles; drains alternate between two engines
so neither ever paces the PE:
  - ACT tiles: scalar-engine EXP with accum_out (exp + row-sum in one
    ACTIVATE, ~2us per megatile).
  - SCH tiles: Schraudolph bit-trick exp on the vector engine --
    code = int32(psum*A + B) is exp(dis) in float bits; one tensor_scalar
    (mult+add, f32->int32) plus one tensor_reduce over the bitcast-f32 tile.
    ~3% deterministic ripple, mean bias corrected on host per column.
Everything O(N) or O(C) runs on host in fp64, so pos/variance are exact.

Sharding: data-parallel over batch N across 8 cores; centers replicated.
Per-class |c_k|^2 is folded as exactly 1.0 into the row bias; the host
applies the mean residual correction (exp(-5*(c2-1)) averaged over C).
"""

import numpy as np
import ml_dtypes

import concourse.bacc as bacc
import concourse.mybir as mybir
import concourse.tile as tile
from concourse.bass_utils import run_bass_kernel_spmd

N, C, D = 8192, 10000, 512
N_CORES = 8
NS = N // N_CORES       # 1024 rows per core
P = 128                 # partitions
M_TILES = NS // P       # 8 row tiles per core
K2 = D // (2 * P)       # 2 DoubleRow contraction tiles (256 rows each)
CW = 512                # matmul free-dim tile (one PSUM bank of fp32)
GW = 2048               # PSUM megatile width: 4 banks
G_TILES = (C + GW - 1) // GW  # 5 (4 x 2048 + 1808)
N_TILES = G_TILES * M_TILES   # 40 megatiles per core
SCALE = 5.0
EPS = 1e-12
FP8_SCALE = 512.0       # 2^9: keeps |values| <= ~120 within e4m3 normal range
FP8 = ml_dtypes.float8_e4m3

# Schraudolph exp in bf16: exp(x) ~= bitcast_bf16(int16(x*SCH_A + SCH_B)).
# int16 codes let the DVE row-sum run at 2x (16-bit) rate.
SCH_A = 128.0 / np.log(2.0)              # 2^7 / ln2
SCH_C = 366393.0 / 65536.0                # minimizes max rel err (~3.5%)
SCH_B = 127.0 * 128.0 - SCH_C

# Megatile drain assignment. ACT tiles: EXP with fused accum row-sum,
# ~2.03us per megatile. SCH tiles: Schraudolph on the DVE -- f32->int16
# convert (2.26us, frees PSUM) + tensor_reduce of the bitcast-bf16 codes
# (2.16us; the DVE gives no 16-bit speedup, ~1 elem/cycle regardless).
# A DVE tile costs 4.42us vs the ~4.1us two-fill PE window, so only every
# third tile goes to the DVE (27 ACT / 13 SCH balances both engines at
# ~55-58us, well under the PE's ~82us). GpSimd cannot help: no PSUM port,
# and the Pool ISA has no tensor_scalar-with-accum.
# Tile 0 (ACT) primes the exp table early; tile 39 (ACT) ends on a fused
# accum so there is no trailing vector work.
ACT_TILES = [t for t in range(N_TILES) if t % 3 != 1]
SCH_TILES = [t for t in range(N_TILES) if t % 3 == 1]
SCH_DVE = {t: i for i, t in enumerate(SCH_TILES)}         # reduce on DVE
ACT_POS = {t: i for i, t in enumerate(ACT_TILES)}
N_ACT_COLS = len(ACT_TILES) + 1          # tile 0 uses two accum slots
N_DVE_COLS = len(SCH_DVE)

_compiled = None
LAST_RESULTS = None


def _build():
    nc = bacc.Bacc(
        "TRN2",
        target_bir_lowering=False,
        debug=False,
        enable_asserts=False,
        num_devices=N_CORES,
    )
    # strip-major: per strip each partition's row is 8 KB contiguous in DRAM;
    # strip 0 is stored as two contiguous half-strips for a fast first fill
    ct0_d = nc.dram_tensor(
        "ct0", [2, P, K2, 2, GW // 2], mybir.dt.float8e4, kind="ExternalInput"
    ).ap()
    ct_d = nc.dram_tensor(
        "ct", [G_TILES - 1, P, K2, 2, GW], mybir.dt.float8e4, kind="ExternalInput"
    ).ap()
    ft_d = nc.dram_tensor(
        "ft", [P, K2, 2, NS], mybir.dt.float8e4, kind="ExternalInput"
    ).ap()
    ab_d = nc.dram_tensor("ab", [P, M_TILES], mybir.dt.float32, kind="ExternalInput").ap()
    # Schraudolph-transformed bias: ab*SCH_A + SCH_B, per row tile
    ab2_d = nc.dram_tensor(
        "ab2", [P, M_TILES], mybir.dt.float32, kind="ExternalInput"
    ).ap()
    # per-(g,m) partial row-sums: ACT partials first, then the DVE ones
    rs_d = nc.dram_tensor(
        "rs",
        [P, N_ACT_COLS + N_DVE_COLS],
        mybir.dt.float32,
        kind="ExternalOutput",
    ).ap()

    with tile.TileContext(nc) as tc:
        with (
            tc.tile_pool(name="cpool", bufs=1) as cpool,
            tc.tile_pool(name="fpool", bufs=1) as fpool,
            tc.tile_pool(name="spool", bufs=1) as spool,
            tc.tile_pool(name="epool", bufs=3) as epool,
            tc.tile_pool(name="ipool", bufs=3) as ipool,
            tc.tile_pool(name="partpool", bufs=1) as partpool,
            tc.tile_pool(name="ppool", bufs=2, space="PSUM") as ppool,
        ):
            # warm the PE clock (HAM) with throwaway DoubleRow matmuls on a
            # zeroed tile while the first input DMAs are still in flight
            z8 = spool.tile([P, 2, CW], mybir.dt.float8e4, tag="z8")
            nc.gpsimd.memset(z8[:], 0.0)
            wps = ppool.tile([P, CW], mybir.dt.float32, tag="ps", name="wps")
            for _ in range(10):
                nc.tensor.matmul(
                    wps[:],
                    z8[:, :, 0:P],
                    z8[:],
                    start=True,
                    stop=True,
                    perf_mode=mybir.MatmulPerfMode.DoubleRow,
                    skip_group_check=True,
                )

            # critical prefix on the fast sync ring, in first-use order
            # (bias is tiny and gates every ACTIVATE - it goes first)
            bias_sb = spool.tile([P, M_TILES], mybir.dt.float32, tag="bias")
            nc.sync.dma_start(out=bias_sb[:], in_=ab_d)
            bias2_sb = spool.tile([P, M_TILES], mybir.dt.float32, tag="bias2")
            nc.sync.dma_start(out=bias2_sb[:], in_=ab2_d)

            ct0a = cpool.tile([P, K2, 2, GW // 2], mybir.dt.float8e4, tag="ct0a")
            nc.sync.dma_start(out=ct0a[:], in_=ct0_d[0])

            ft_sb = fpool.tile([P, K2, 2, NS], mybir.dt.float8e4, tag="ft")
            nc.sync.dma_start(out=ft_sb[:], in_=ft_d)

            ct0b = cpool.tile([P, K2, 2, GW // 2], mybir.dt.float8e4, tag="ct0b")
            nc.sync.dma_start(out=ct0b[:], in_=ct0_d[1])

            # remaining strips: one DMA per strip, all FIFO on the sync ring
            # so late strips never contend with the critical prefix
            ct_sb = [None]
            for g in range(1, G_TILES):
                gw = min(GW, C - g * GW)
                t = cpool.tile(
                    [P, K2, 2, GW], mybir.dt.float8e4, tag=f"ct{g}", name=f"ct{g}"
                )
                nc.sync.dma_start(out=t[:, :, :, :gw], in_=ct_d[g - 1][:, :, :, :gw])
                ct_sb.append(t)

            parts_act = partpool.tile([P, N_ACT_COLS], mybir.dt.float32, tag="pa")
            parts_dve = partpool.tile([P, N_DVE_COLS], mybir.dt.float32, tag="pd")

            # strip-outer / row-tile-inner: PE is dense as soon as strip 0 lands
            for g in range(G_TILES):
                gw = min(GW, C - g * GW)
                n_sl = (gw + CW - 1) // CW
                for m in range(M_TILES):
                    tile_idx = g * M_TILES + m
                    on_act = tile_idx in ACT_POS
                    ps = ppool.tile([P, GW], mybir.dt.float32, tag="ps")
                    # first megatile: finish columns in halves (j-outer) and
                    # exp each half as it lands, so ACT primes ~3us earlier
                    split_act = tile_idx == 0
                    kj = (
                        [(k, j) for j in range(n_sl) for k in range(K2)]
                        if split_act
                        else [(k, j) for k in range(K2) for j in range(n_sl)]
                    )
                    for k, j in kj:
                        w = min(CW, gw - j * CW)
                        if g == 0:
                            half = (j * CW) // (GW // 2)
                            off = (j * CW) % (GW // 2)
                            rhs = (ct0a if half == 0 else ct0b)[
                                :, k, :, off : off + w
                            ]
                        else:
                            rhs = ct_sb[g][:, k, :, j * CW : j * CW + w]
                        nc.tensor.matmul(
                            ps[:, j * CW : j * CW + w],
                            ft_sb[:, k, :, m * P : (m + 1) * P],
                            rhs,
                            start=(k == 0),
                            stop=(k == K2 - 1),
                            perf_mode=mybir.MatmulPerfMode.DoubleRow,
                            skip_group_check=True,
                        )
                    if split_act:
                        # ACT path, two halves, each with its own accum slot
                        et = epool.tile([P, GW], mybir.dt.bfloat16, tag="exp")
                        h = GW // 2
                        for hi in range(2):
                            nc.scalar.activation(
                                et[:, hi * h : (hi + 1) * h],
                                ps[:, hi * h : (hi + 1) * h],
                                mybir.ActivationFunctionType.Exp,
                                bias=bias_sb[:, m : m + 1],
                                scale=2.0 * SCALE / (FP8_SCALE * FP8_SCALE),
                                accum_out=parts_act[:, hi : hi + 1],
                            )
                    elif on_act:
                        ci = ACT_POS[tile_idx] + 1
                        et = epool.tile([P, GW], mybir.dt.bfloat16, tag="exp")
                        nc.scalar.activation(
                            et[:, :gw],
                            ps[:, :gw],
                            mybir.ActivationFunctionType.Exp,
                            bias=bias_sb[:, m : m + 1],
                            scale=2.0 * SCALE / (FP8_SCALE * FP8_SCALE),
                            accum_out=parts_act[:, ci : ci + 1],
                        )
                    else:
                        # Schraudolph on DVE: int16(psum*A' + B_m) = exp in
                        # bf16 bits, then row-sum of the bitcast-bf16 codes
                        it = ipool.tile([P, GW], mybir.dt.int16, tag="icode")
                        nc.vector.tensor_scalar(
                            it[:, :gw],
                            ps[:, :gw],
                            float(SCH_A * 2.0 * SCALE / (FP8_SCALE * FP8_SCALE)),
                            bias2_sb[:, m : m + 1],
                            op0=mybir.AluOpType.mult,
                            op1=mybir.AluOpType.add,
                        )
                        ci = SCH_DVE[tile_idx]
                        nc.vector.tensor_reduce(
                            parts_dve[:, ci : ci + 1],
                            it[:, :gw].bitcast(mybir.dt.bfloat16),
                            axis=mybir.AxisListType.X,
                            op=mybir.AluOpType.add,
                        )
            nc.sync.dma_start(out=rs_d[:, 0:N_ACT_COLS], in_=parts_act[:])
            nc.sync.dma_start(out=rs_d[:, N_ACT_COLS:], in_=parts_dve[:])

    nc.compile()
    return nc


def _get_compiled():
    global _compiled
    if _compiled is None:
        _compiled = _build()
    return _compiled


def _l2n(x):
    n = np.sqrt(np.einsum("nd,nd->n", x, x, dtype=np.float32), dtype=np.float32)
    xh = x / np.maximum(n, np.float32(EPS))[:, None]
    sq = np.einsum("nd,nd->n", xh, xh, dtype=np.float32)
    return xh.astype(np.float32), sq.astype(np.float32)


def _pack_dr(xt):
    """[D, W] fp32 (pre-scaled) -> DoubleRow fp8 [P, K2, 2, W]:
    row d = k*256 + i*128 + p  ->  out[p, k, i]."""
    d, w = xt.shape
    return np.ascontiguousarray(
        xt.reshape(K2, 2, P, w).transpose(2, 0, 1, 3)
    ).astype(FP8)


def _pack_ct(xt):
    """[D, C] fp32 (pre-scaled) -> (strip-0 halves [2, P, K2, 2, GW/2],
    strips 1.. [G-1, P, K2, 2, GW], last zero-padded)."""
    ct0 = np.stack(
        [_pack_dr(xt[:, 0 : GW // 2]), _pack_dr(xt[:, GW // 2 : GW])]
    )
    ctr = np.zeros((G_TILES - 1, P, K2, 2, GW), dtype=FP8)
    for g in range(1, G_TILES):
        gw = min(GW, C - g * GW)
        ctr[g - 1, :, :, :, :gw] = _pack_dr(xt[:, g * GW : g * GW + gw])
    return ct0, ctr


def _sch_emulate(x):
    """Numpy emulation of the device Schraudolph path for fp32 input x.
    The DVE's f32->int16 output conversion rounds to nearest."""
    y = np.float32(x) * np.float32(SCH_A) + np.float32(SCH_B)
    code = np.rint(y).astype(np.int16)
    return code.view(ml_dtypes.bfloat16).astype(np.float32)


def _sch_mean_corr():
    """Mean multiplicative bias of the Schraudolph approx over a uniform
    phase (inputs spread over many ln2 periods), to divide out on host."""
    x = np.linspace(-12.0, -12.0 + np.log(2.0), 65537, dtype=np.float64)[:-1]
    ratio = _sch_emulate(x.astype(np.float32)).astype(np.float64) / np.exp(x)
    return ratio.mean()


SCH_CORR = 1.0 / _sch_mean_corr()


def _combine_rs(rs):
    """[P, N_ACT_COLS+N_GPS_COLS+N_DVE_COLS] per-core output -> per-row sums
    [NS] (n = m*128 + p). Applies the Schraudolph mean-bias correction to SCH
    columns and re-interleaves storage order back to tile order."""
    rs = rs.astype(np.float64)
    vals = np.empty((P, N_TILES), dtype=np.float64)
    # tile 0 = sum of the two half-accums
    vals[:, 0] = rs[:, 0] + rs[:, 1]
    for t in ACT_TILES[1:]:
        vals[:, t] = rs[:, ACT_POS[t] + 1]
    for t, i in SCH_DVE.items():
        vals[:, t] = rs[:, N_ACT_COLS + i] * SCH_CORR
    out = vals.reshape(P, G_TILES, M_TILES).sum(axis=1)
    return out.T.reshape(NS)


def kernel(features, labels, centers, bias):
    features = np.asarray(features, dtype=np.float32)
    centers = np.asarray(centers, dtype=np.float32)
    bias = np.asarray(bias, dtype=np.float32)
    labels_i = np.asarray(labels).astype(np.int64)

    fh, f2 = _l2n(features)          # [N, D], [N]
    ch, c2 = _l2n(centers)           # [C, D], [C]

    ct0_8, ct8 = _pack_ct(ch.T * np.float32(FP8_SCALE))
    abias_full = (-SCALE * (f2 + np.float32(1.0))).astype(np.float32)
    ab2_full = (
        abias_full.astype(np.float64) * SCH_A + SCH_B
    ).astype(np.float32)

    in_maps = []
    for i in range(N_CORES):
        sl = slice(i * NS, (i + 1) * NS)
        ft8 = _pack_dr(fh[sl].T * np.float32(FP8_SCALE))    # [P, K2, 2, NS]
        ab = np.ascontiguousarray(
            abias_full[sl].reshape(M_TILES, P).T
        )  # [P, M_TILES], n = m*128 + p
        ab2 = np.ascontiguousarray(ab2_full[sl].reshape(M_TILES, P).T)
        in_maps.append({"ct0": ct0_8, "ct": ct8, "ft": ft8, "ab": ab, "ab2": ab2})

    nc = _get_compiled()
    global LAST_RESULTS
    LAST_RESULTS = run_bass_kernel_spmd(nc, in_maps, core_ids=list(range(N_CORES)))

    rowsum = np.concatenate(
        [_combine_rs(LAST_RESULTS.results[i]["rs"]) for i in range(N_CORES)]
    ).astype(np.float64)

    # residual correction for the |c_k|^2 ~= 1 fold (mean of exp(-5*(c2-1)))
    wmean = np.exp(-SCALE * (c2.astype(np.float64) - 1.0)).mean()
    rowsum *= wmean

    # exact per-row label terms (fp32 inputs, fp64 math)
    cl = ch[labels_i]                                        # [N, D]
    dot = np.einsum("nd,nd->n", fh.astype(np.float64), cl.astype(np.float64))
    dis_l = -SCALE * (f2.astype(np.float64) + c2[labels_i].astype(np.float64) - 2.0 * dot)
    pos = dis_l + bias[labels_i, 0].astype(np.float64)

    num = np.exp(pos)
    den = rowsum - np.exp(dis_l) + num
    logits = np.log(den) - pos
    variance = np.var(pos, ddof=1)
    loss = logits.mean() + variance
    return (np.float32(loss), np.float32(variance))


# revision 24
# speedup vs baseline: 1.2332x; 1.0109x over previous
"""Trainium2 Bass kernel for nn_CenterLossNet (center-loss softmax over classes).

Math (reference):
    f = l2_normalize(features); c = l2_normalize(centers)
    dis[n,k]  = -5 * (|f_n|^2 + |c_k|^2 - 2 f_n.c_k)        # [N, C]
    pos[n]    = dis[n, labels[n]] + bias[labels[n]]
    den[n]    = sum_k exp(dis[n,k]) - exp(dis[n,l_n]) + exp(pos[n])
    loss      = mean(log(den) - pos) + var(pos, ddof=1);  returns (loss, var)

Device does the heavy part: S = f_hat @ c_hat.T (8192x10000x512 matmul) in
fp8e4m3 DoubleRow perf mode, fused with exp + row-sum of exp(10*S + bias_n).
The PE fills 2048-wide PSUM megatiles; drains alternate between two engines
so neither ever paces the PE:
  - ACT tiles: scalar-engine EXP with accum_out (exp + row-sum in one
    ACTIVATE, ~2us per megatile).
  - SCH tiles: Schraudolph bit-trick exp on the vector engine --
    code = int32(psum*A + B) is exp(dis) in float bits; one tensor_scalar
    (mult+add, f32->int32) plus one tensor_reduce over the bitcast-f32 tile.
    ~3% deterministic ripple, mean bias corrected on host per column.
Everything O(N) or O(C) runs on host in fp64, so pos/variance are exact.

Sharding: data-parallel over batch N across 8 cores; centers replicated.
Per-class |c_k|^2 is folded as exactly 1.0 into the row bias; the host
applies the mean residual correction (exp(-5*(c2-1)) averaged over C).
"""

import numpy as np
import ml_dtypes

import concourse.bacc as bacc
import concourse.mybir as mybir
import concourse.tile as tile
from concourse.bass_utils import run_bass_kernel_spmd

N, C, D = 8192, 10000, 512
N_CORES = 8
NS = N // N_CORES       # 1024 rows per core
P = 128                 # partitions
M_TILES = NS // P       # 8 row tiles per core
K2 = D // (2 * P)       # 2 DoubleRow contraction tiles (256 rows each)
CW = 512                # matmul free-dim tile (one PSUM bank of fp32)
GW = 2048               # PSUM megatile width: 4 banks
G_TILES = (C + GW - 1) // GW  # 5 (4 x 2048 + 1808)
N_TILES = G_TILES * M_TILES   # 40 megatiles per core
SCALE = 5.0
EPS = 1e-12
FP8_SCALE = 512.0       # 2^9: keeps |values| <= ~120 within e4m3 normal range
FP8 = ml_dtypes.float8_e4m3

# Schraudolph exp in bf16: exp(x) ~= bitcast_bf16(int16(x*SCH_A + SCH_B)).
# int16 codes let the DVE row-sum run at 2x (16-bit) rate.
SCH_A = 128.0 / np.log(2.0)              # 2^7 / ln2
SCH_C = 366393.0 / 65536.0                # minimizes max rel err (~3.5%)
SCH_B = 127.0 * 128.0 - SCH_C

# Megatile drain assignment. ACT tiles: EXP with fused accum row-sum,
# ~2.03us per megatile. SCH tiles: Schraudolph on the DVE -- f32->int16
# convert (2.26us, frees PSUM) + tensor_reduce of the bitcast-bf16 codes
# (2.16us; the DVE gives no 16-bit speedup, ~1 elem/cycle regardless).
# A DVE tile costs 4.42us vs the ~4.1us two-fill PE window, so only every
# third tile goes to the DVE (27 ACT / 13 SCH balances both engines at
# ~55-58us, well under the PE's ~82us). GpSimd cannot help: no PSUM port,
# and the Pool ISA has no tensor_scalar-with-accum.
# Tile 0 (ACT) primes the exp table early; tile 39 (ACT) ends on a fused
# accum so there is no trailing vector work.
ACT_TILES = [t for t in range(N_TILES) if t % 3 != 1]
SCH_TILES = [t for t in range(N_TILES) if t % 3 == 1]
SCH_DVE = {t: i for i, t in enumerate(SCH_TILES)}         # reduce on DVE
ACT_POS = {t: i for i, t in enumerate(ACT_TILES)}
# tiles 0 and 39 drain in two halves (early exp-table prime / shorter tail),
# each half with its own accum slot: cols [2i, 2i+1]; the other ACT tiles
# get one col each starting at 2*len(SPLIT_TILES)
SPLIT_TILES = (0, N_TILES - 1)
NONSPLIT_ACT = {
    t: i for i, t in enumerate(t for t in ACT_TILES if t not in SPLIT_TILES)
}
N_ACT_COLS = 2 * len(SPLIT_TILES) + len(NONSPLIT_ACT)
N_DVE_COLS = len(SCH_DVE)

_compiled = None
LAST_RESULTS = None


def _build():
    nc = bacc.Bacc(
        "TRN2",
        target_bir_lowering=False,
        debug=False,
        enable_asserts=False,
        num_devices=N_CORES,
    )
    # strip-major: per strip each partition's row is 8 KB contiguous in DRAM;
    # strip 0 is stored as four contiguous quarter-strips (one per 512-col
    # matmul slice) so the first matmul can start after only 256 KB lands
    ct0_d = nc.dram_tensor(
        "ct0", [4, P, K2, 2, GW // 4], mybir.dt.float8e4, kind="ExternalInput"
    ).ap()
    ct_d = nc.dram_tensor(
        "ct", [G_TILES - 1, P, K2, 2, GW], mybir.dt.float8e4, kind="ExternalInput"
    ).ap()
    ft_d = nc.dram_tensor(
        "ft", [P, K2, 2, NS], mybir.dt.float8e4, kind="ExternalInput"
    ).ap()
    ab_d = nc.dram_tensor("ab", [P, M_TILES], mybir.dt.float32, kind="ExternalInput").ap()
    # Schraudolph-transformed bias: ab*SCH_A + SCH_B, per row tile
    ab2_d = nc.dram_tensor(
        "ab2", [P, M_TILES], mybir.dt.float32, kind="ExternalInput"
    ).ap()
    # per-(g,m) partial row-sums: ACT partials first, then the DVE ones
    rs_d = nc.dram_tensor(
        "rs",
        [P, N_ACT_COLS + N_DVE_COLS],
        mybir.dt.float32,
        kind="ExternalOutput",
    ).ap()

    with tile.TileContext(nc) as tc:
        with (
            tc.tile_pool(name="cpool", bufs=1) as cpool,
            tc.tile_pool(name="fpool", bufs=1) as fpool,
            tc.tile_pool(name="spool", bufs=1) as spool,
            tc.tile_pool(name="epool", bufs=3) as epool,
            tc.tile_pool(name="ipool", bufs=3) as ipool,
            tc.tile_pool(name="partpool", bufs=1) as partpool,
            tc.tile_pool(name="ppool", bufs=2, space="PSUM") as ppool,
        ):
            # warm the PE clock (HAM) with throwaway DoubleRow matmuls on a
            # zeroed tile while the first input DMAs are still in flight
            z8 = spool.tile([P, 2, CW], mybir.dt.float8e4, tag="z8")
            nc.gpsimd.memset(z8[:], 0.0)
            wps = ppool.tile([P, CW], mybir.dt.float32, tag="ps", name="wps")
            for _ in range(10):
                nc.tensor.matmul(
                    wps[:],
                    z8[:, :, 0:P],
                    z8[:],
                    start=True,
                    stop=True,
                    perf_mode=mybir.MatmulPerfMode.DoubleRow,
                    skip_group_check=True,
                )

            # critical prefix on the fast sync ring, in first-use order
            # (bias is tiny and gates every ACTIVATE - it goes first)
            bias_sb = spool.tile([P, M_TILES], mybir.dt.float32, tag="bias")
            nc.sync.dma_start(out=bias_sb[:], in_=ab_d)
            bias2_sb = spool.tile([P, M_TILES], mybir.dt.float32, tag="bias2")
            nc.sync.dma_start(out=bias2_sb[:], in_=ab2_d)

            ct0q = []
            for q in range(4):
                t = cpool.tile(
                    [P, K2, 2, GW // 4], mybir.dt.float8e4, tag=f"ct0q{q}"
                )
                nc.sync.dma_start(out=t[:], in_=ct0_d[q])
                ct0q.append(t)
                if q == 0:
                    ft_sb = fpool.tile(
                        [P, K2, 2, NS], mybir.dt.float8e4, tag="ft"
                    )
                    nc.sync.dma_start(out=ft_sb[:], in_=ft_d)

            # remaining strips: one DMA per strip, all FIFO on the sync ring
            # so late strips never contend with the critical prefix
            ct_sb = [None]
            for g in range(1, G_TILES):
                gw = min(GW, C - g * GW)
                t = cpool.tile(
                    [P, K2, 2, GW], mybir.dt.float8e4, tag=f"ct{g}", name=f"ct{g}"
                )
                nc.sync.dma_start(out=t[:, :, :, :gw], in_=ct_d[g - 1][:, :, :, :gw])
                ct_sb.append(t)

            parts_act = partpool.tile([P, N_ACT_COLS], mybir.dt.float32, tag="pa")
            parts_dve = partpool.tile([P, N_DVE_COLS], mybir.dt.float32, tag="pd")

            # strip-outer / row-tile-inner: PE is dense as soon as strip 0 lands
            for g in range(G_TILES):
                gw = min(GW, C - g * GW)
                n_sl = (gw + CW - 1) // CW
                for m in range(M_TILES):
                    tile_idx = g * M_TILES + m
                    on_act = tile_idx in ACT_POS
                    ps = ppool.tile([P, GW], mybir.dt.float32, tag="ps")
                    # j-outer / k-inner: each 512-col slice finishes before
                    # the next starts, so drains on earlier columns can begin
                    # while later columns are still multiplying (tile deps
                    # are AP-range granular)
                    for j in range(n_sl):
                        w = min(CW, gw - j * CW)
                        for k in range(K2):
                            rhs = (
                                ct0q[j][:, k, :, :w]
                                if g == 0
                                else ct_sb[g][:, k, :, j * CW : j * CW + w]
                            )
                            nc.tensor.matmul(
                                ps[:, j * CW : j * CW + w],
                                ft_sb[:, k, :, m * P : (m + 1) * P],
                                rhs,
                                start=(k == 0),
                                stop=(k == K2 - 1),
                                perf_mode=mybir.MatmulPerfMode.DoubleRow,
                                skip_group_check=True,
                            )
                    if on_act:
                        et = epool.tile([P, GW], mybir.dt.bfloat16, tag="exp")
                        if tile_idx in SPLIT_TILES:
                            # two halves, each with its own accum slot: the
                            # first EXP starts while the second half is still
                            # multiplying (early table prime / shorter tail)
                            base = 2 * SPLIT_TILES.index(tile_idx)
                            h = GW // 2
                            for hi in range(2):
                                w2 = min(h, gw - hi * h)
                                nc.scalar.activation(
                                    et[:, hi * h : hi * h + w2],
                                    ps[:, hi * h : hi * h + w2],
                                    mybir.ActivationFunctionType.Exp,
                                    bias=bias_sb[:, m : m + 1],
                                    scale=2.0 * SCALE / (FP8_SCALE * FP8_SCALE),
                                    accum_out=parts_act[:, base + hi : base + hi + 1],
                                )
                        else:
                            ci = 2 * len(SPLIT_TILES) + NONSPLIT_ACT[tile_idx]
                            nc.scalar.activation(
                                et[:, :gw],
                                ps[:, :gw],
                                mybir.ActivationFunctionType.Exp,
                                bias=bias_sb[:, m : m + 1],
                                scale=2.0 * SCALE / (FP8_SCALE * FP8_SCALE),
                                accum_out=parts_act[:, ci : ci + 1],
                            )
                    else:
                        # Schraudolph on DVE: int16(psum*A' + B_m) = exp in
                        # bf16 bits. Convert in two halves so PSUM banks free
                        # early; then one row-sum of the bitcast-bf16 codes.
                        it = ipool.tile([P, GW], mybir.dt.int16, tag="icode")
                        h = GW // 2
                        for hi in range(2):
                            w2 = min(h, gw - hi * h)
                            nc.vector.tensor_scalar(
                                it[:, hi * h : hi * h + w2],
                                ps[:, hi * h : hi * h + w2],
                                float(SCH_A * 2.0 * SCALE / (FP8_SCALE * FP8_SCALE)),
                                bias2_sb[:, m : m + 1],
                                op0=mybir.AluOpType.mult,
                                op1=mybir.AluOpType.add,
                            )
                        ci = SCH_DVE[tile_idx]
                        nc.vector.tensor_reduce(
                            parts_dve[:, ci : ci + 1],
                            it[:, :gw].bitcast(mybir.dt.bfloat16),
                            axis=mybir.AxisListType.X,
                            op=mybir.AluOpType.add,
                        )
            nc.sync.dma_start(out=rs_d[:, 0:N_ACT_COLS], in_=parts_act[:])
            nc.sync.dma_start(out=rs_d[:, N_ACT_COLS:], in_=parts_dve[:])

    nc.compile()
    return nc


def _get_compiled():
    global _compiled
    if _compiled is None:
        _compiled = _build()
    return _compiled


def _l2n(x):
    n = np.sqrt(np.einsum("nd,nd->n", x, x, dtype=np.float32), dtype=np.float32)
    xh = x / np.maximum(n, np.float32(EPS))[:, None]
    sq = np.einsum("nd,nd->n", xh, xh, dtype=np.float32)
    return xh.astype(np.float32), sq.astype(np.float32)


def _pack_dr(xt):
    """[D, W] fp32 (pre-scaled) -> DoubleRow fp8 [P, K2, 2, W]:
    row d = k*256 + i*128 + p  ->  out[p, k, i]."""
    d, w = xt.shape
    return np.ascontiguousarray(
        xt.reshape(K2, 2, P, w).transpose(2, 0, 1, 3)
    ).astype(FP8)


def _pack_ct(xt):
    """[D, C] fp32 (pre-scaled) -> (strip-0 quarters [4, P, K2, 2, GW/4],
    strips 1.. [G-1, P, K2, 2, GW], last zero-padded)."""
    q = GW // 4
    ct0 = np.stack([_pack_dr(xt[:, i * q : (i + 1) * q]) for i in range(4)])
    ctr = np.zeros((G_TILES - 1, P, K2, 2, GW), dtype=FP8)
    for g in range(1, G_TILES):
        gw = min(GW, C - g * GW)
        ctr[g - 1, :, :, :, :gw] = _pack_dr(xt[:, g * GW : g * GW + gw])
    return ct0, ctr


def _sch_emulate(x):
    """Numpy emulation of the device Schraudolph path for fp32 input x.
    The DVE's f32->int16 output conversion rounds to nearest."""
    y = np.float32(x) * np.float32(SCH_A) + np.float32(SCH_B)
    code = np.rint(y).astype(np.int16)
    return code.view(ml_dtypes.bfloat16).astype(np.float32)


def _sch_mean_corr():
    """Mean multiplicative bias of the Schraudolph approx over a uniform
    phase (inputs spread over many ln2 periods), to divide out on host."""
    x = np.linspace(-12.0, -12.0 + np.log(2.0), 65537, dtype=np.float64)[:-1]
    ratio = _sch_emulate(x.astype(np.float32)).astype(np.float64) / np.exp(x)
    return ratio.mean()


SCH_CORR = 1.0 / _sch_mean_corr()


def _combine_rs(rs):
    """[P, N_ACT_COLS+N_GPS_COLS+N_DVE_COLS] per-core output -> per-row sums
    [NS] (n = m*128 + p). Applies the Schraudolph mean-bias correction to SCH
    columns and re-interleaves storage order back to tile order."""
    rs = rs.astype(np.float64)
    vals = np.empty((P, N_TILES), dtype=np.float64)
    for i, t in enumerate(SPLIT_TILES):
        vals[:, t] = rs[:, 2 * i] + rs[:, 2 * i + 1]
    for t, i in NONSPLIT_ACT.items():
        vals[:, t] = rs[:, 2 * len(SPLIT_TILES) + i]
    for t, i in SCH_DVE.items():
        vals[:, t] = rs[:, N_ACT_COLS + i] * SCH_CORR
    out = vals.reshape(P, G_TILES, M_TILES).sum(axis=1)
    return out.T.reshape(NS)


def kernel(features, labels, centers, bias):
    features = np.asarray(features, dtype=np.float32)
    centers = np.asarray(centers, dtype=np.float32)
    bias = np.asarray(bias, dtype=np.float32)
    labels_i = np.asarray(labels).astype(np.int64)

    fh, f2 = _l2n(features)          # [N, D], [N]
    ch, c2 = _l2n(centers)           # [C, D], [C]

    ct0_8, ct8 = _pack_ct(ch.T * np.float32(FP8_SCALE))
    abias_full = (-SCALE * (f2 + np.float32(1.0))).astype(np.float32)
    ab2_full = (
        abias_full.astype(np.float64) * SCH_A + SCH_B
    ).astype(np.float32)

    in_maps = []
    for i in range(N_CORES):
        sl = slice(i * NS, (i + 1) * NS)
        ft8 = _pack_dr(fh[sl].T * np.float32(FP8_SCALE))    # [P, K2, 2, NS]
        ab = np.ascontiguousarray(
            abias_full[sl].reshape(M_TILES, P).T
        )  # [P, M_TILES], n = m*128 + p
        ab2 = np.ascontiguousarray(ab2_full[sl].reshape(M_TILES, P).T)
        in_maps.append({"ct0": ct0_8, "ct": ct8, "ft": ft8, "ab": ab, "ab2": ab2})

    nc = _get_compiled()
    global LAST_RESULTS
    LAST_RESULTS = run_bass_kernel_spmd(nc, in_maps, core_ids=list(range(N_CORES)))

    rowsum = np.concatenate(
        [_combine_rs(LAST_RESULTS.results[i]["rs"]) for i in range(N_CORES)]
    ).astype(np.float64)

    # residual correction for the |c_k|^2 ~= 1 fold (mean of exp(-5*(c2-1)))
    wmean = np.exp(-SCALE * (c2.astype(np.float64) - 1.0)).mean()
    rowsum *= wmean

    # exact per-row label terms (fp32 inputs, fp64 math)
    cl = ch[labels_i]                                        # [N, D]
    dot = np.einsum("nd,nd->n", fh.astype(np.float64), cl.astype(np.float64))
    dis_l = -SCALE * (f2.astype(np.float64) + c2[labels_i].astype(np.float64) - 2.0 * dot)
    pos = dis_l + bias[labels_i, 0].astype(np.float64)

    num = np.exp(pos)
    den = rowsum - np.exp(dis_l) + num
    logits = np.log(den) - pos
    variance = np.var(pos, ddof=1)
    loss = logits.mean() + variance
    return (np.float32(loss), np.float32(variance))
